# revision 1
# baseline (speedup 1.0000x reference)
"""Trainium2 Bass kernel for 8-head MultiHeadAttention (B=2, S=4096, E=512).

Sharding: 8 cores = 2 batches x 4 query-row chunks of 1024; each core runs
all 8 heads for its (batch, q-range) as 4 head-pairs x 2 query-windows, with
the k-dimension processed in 32 chunks of 128 ("half-units" of one head x
512 q x 128 k).

Engine-balanced softmax-attention pipeline (~65-82% busy on all of PE/ACT/
DVE):
- Q/K projected on-chip (bf16); scores are plain bf16 matmuls [128k, 512q].
- The exp work is split across engines: ~20/32 k-chunks use the ACT table
  exp; 12/32 use a Schraudolph bit-trick exp on DVE - ONE scalar_tensor_
  tensor computes i16 = round((s + 704) * (a16 * mask)) whose bits ARE the
  bf16 weights (mask folded in as the multiplicand, a16 calibrated so the
  mean weight ratio vs exact exp is 1; ~1.8% sawtooth RMS on 38% of chunks
  keeps total output error at ~1.2e-2).
- ACT-chunk masks are applied pre-exp on the PE as fp8 DoubleRow
  identity-matmuls (adds 240*mask into PSUM at 0.5 cycles/row; the exp bias
  -30 shifts masked scores to ~e^-30) or post-exp on DVE (bf16 2x multiply).
- The attention-value matmul is flipped: pt [128k,128q] is the STATIONARY
  operand and raw V+ones [128,65] moves -> ctx [128q, 65] at 65 cycles per
  matmul (4x fewer PE cycles than the [65, 512] orientation; ldweights are
  free). The ones column lands the softmax denominator per PARTITION, so
  normalization is a cheap per-partition tensor_scalar.
- cx accumulators are bank-aligned and only the first block per 2KB PSUM
  bank sets start=True (a matmul's start zeroes the whole 2KB region).
- Wv is folded into Wo on the host (Wo_eff = Wo . blockdiag(Wv)); the
  normalized ctx is transposed on PE (64x128 transposes through the shared
  PSUM ring) straight into the output projection.
- Single shared PSUM ring ([128,512] x6 banks) holds score tiles, projection
  chunks, transposes and the output projection; software pipelining runs the
  attention-value matmuls 8 half-units behind the scores so their pt
  dependencies never stall the in-order PE queue.
"""
import sys
for _p in ('/root/.axon_site/_ro/trn_rl_repo', '/opt/trn_rl_repo'):
    if _p not in sys.path:
        sys.path.append(_p)

import numpy as np
import ml_dtypes

import concourse.bass as bass
import concourse.tile as tile
from concourse import bacc, mybir
from concourse import bass_utils

F32 = mybir.dt.float32
BF16 = mybir.dt.bfloat16
FP8 = mybir.dt.float8e4
I16 = mybir.dt.int16
AF = mybir.ActivationFunctionType
ALU = mybir.AluOpType
DR = mybir.MatmulPerfMode.DoubleRow

N_CORES = 8
B, S, E, H, DH = 2, 4096, 512, 8, 64
QLEN = S // 4           # 1024 q rows per core
KC = S // 128           # 32 k chunks

BBIG = 704.0            # 22*32, fp8-exact, added to every score via row 32


def _calib_a16():
    """Schraudolph multiplier: bf16_bits = (s + 704) * a16. Calibrate so the
    mean weight ratio vs exact exp(s/8) over s~N(0,8^2) is 1 (the residual
    sawtooth then largely cancels between numerator and denominator)."""
    base = 0.125 * np.log2(np.e) * 128.0
    s = np.linspace(-40, 40, 20001)
    w = np.exp(-0.5 * (s / 8.0) ** 2)

    def ratio(a16):
        bits = np.round((s + BBIG) * a16).astype(np.int64)
        vals = np.frombuffer(
            bits.astype(np.int16).tobytes(), dtype=ml_dtypes.bfloat16
        ).astype(np.float64)
        exact = np.exp(s / 8.0)
        return np.average(vals / exact, weights=w)

    lo, hi = base * 0.9995, base * 1.0005
    for _ in range(40):
        mid = 0.5 * (lo + hi)
        if ratio(mid) > 1.0:
            hi = mid
        else:
            lo = mid
    return 0.5 * (lo + hi)


A16 = float(_calib_a16())

# per-kc class: 'dve' = Schraudolph on DVE; others use ACT exp with the mask
# applied on PE (DR-identity), Pool, or DVE (bf16 tensor mult).
CLS = {}
_DVE_KCS = {1, 4, 6, 9, 12, 14, 17, 20, 22, 25, 28, 30}
_PE_KCS = {0, 3, 5, 7, 10, 11, 13, 15, 16, 19, 21, 23, 26, 27, 29, 31}
_DVM_KCS_ = {2, 8, 18, 24}
for _kc in range(KC):
    if _kc in _DVE_KCS:
        CLS[_kc] = 'dve'
    elif _kc in _PE_KCS:
        CLS[_kc] = 'pe'
    elif _kc in _DVM_KCS_:
        CLS[_kc] = 'dvm'
    else:
        CLS[_kc] = 'pool'
DVE_KCS = sorted(k for k, v in CLS.items() if v == 'dve')
PE_KCS = sorted(k for k, v in CLS.items() if v == 'pe')
BF_KCS = sorted(k for k, v in CLS.items() if v in ('dvm', 'pool'))
F16 = mybir.dt.float16

# proj psum->sbuf copy engine by (global copy index)
def _copy_engine(idx):
    return 'v' if idx % 4 < 3 else 'a'

_CACHE = {}
DEBUG = False


def _build_module():
    nc = bacc.Bacc("TRN2", target_bir_lowering=False, debug=False,
                   enable_asserts=True, num_devices=N_CORES)

    xkT = nc.dram_tensor("xkT", [E, S], BF16, kind="ExternalInput").ap()
    xqT = nc.dram_tensor("xqT", [E, QLEN], BF16, kind="ExternalInput").ap()
    valp = nc.dram_tensor("valp", [S, H * 65], BF16, kind="ExternalInput").ap()
    mbq = nc.dram_tensor("mbq", [S, QLEN], FP8, kind="ExternalInput").ap()
    mbb = nc.dram_tensor("mbb", [S, QLEN], BF16, kind="ExternalInput").ap()
    mba = nc.dram_tensor("mba", [S, QLEN], F16, kind="ExternalInput").ap()
    wk2 = nc.dram_tensor("wk2", [128, DH], BF16, kind="ExternalInput").ap()
    wq2 = nc.dram_tensor("wq2", [128, DH], BF16, kind="ExternalInput").ap()
    woe = nc.dram_tensor("woe", [E, E], BF16, kind="ExternalInput").ap()
    bo_b = nc.dram_tensor("bo_b", [128, E], F32, kind="ExternalInput").ap()
    eye_d = nc.dram_tensor("eye", [128, 128], BF16, kind="ExternalInput").ap()
    i2_d = nc.dram_tensor("i2", [128, 256], FP8, kind="ExternalInput").ap()
    out = nc.dram_tensor("out", [QLEN, E], F32, kind="ExternalOutput").ap()
    dbg = {}
    if DEBUG:
        dbg['kp2'] = nc.dram_tensor("dkp2", [128, 8192], FP8, kind="ExternalOutput").ap()
        dbg['qp2'] = nc.dram_tensor("dqp2", [128, 2048], FP8, kind="ExternalOutput").ap()
        dbg['pt0'] = nc.dram_tensor("dpt0", [128, 512], F32, kind="ExternalOutput").ap()
        dbg['pt1'] = nc.dram_tensor("dpt1", [128, 512], F32, kind="ExternalOutput").ap()
        dbg['cx'] = nc.dram_tensor("dcx", [128, 1024], F32, kind="ExternalOutput").ap()
        dbg['ct'] = nc.dram_tensor("dct", [128, 1024], F32, kind="ExternalOutput").ap()

    with tile.TileContext(nc) as tc:
        _emit(tc, nc, xkT, xqT, valp, mbq, mbb, mba, wk2, wq2, woe, bo_b,
              eye_d, i2_d, out, dbg)

    nc.compile()
    return nc


def _emit(tc, nc, xkT, xqT, valp, mbq, mbb, mba, wk2, wq2, woe, bo_b,
          eye_d, i2_d, out, dbg={}):
    from contextlib import ExitStack
    ctx = ExitStack()
    const = ctx.enter_context(tc.tile_pool(name="const", bufs=1))
    kqp = ctx.enter_context(tc.tile_pool(name="kqp", bufs=1))
    xst = ctx.enter_context(tc.tile_pool(name="xst", bufs=2))
    ptp = ctx.enter_context(tc.tile_pool(name="pt", bufs=12))
    ctn_p = ctx.enter_context(tc.tile_pool(name="ctn", bufs=20))
    osb_p = ctx.enter_context(tc.tile_pool(name="osb", bufs=4))
    dbg_p = ctx.enter_context(tc.tile_pool(name="dbgp", bufs=1)) if dbg else None
    psp = ctx.enter_context(tc.tile_pool(name="psp", bufs=6, space="PSUM"))
    ctxp = ctx.enter_context(tc.tile_pool(name="ctxp", bufs=1, space="PSUM"))

    # ---------------- constants ----------------
    wk_sb = const.tile([128, DH], BF16, tag="wk")
    nc.sync.dma_start(wk_sb, wk2)
    wq_sb = const.tile([128, DH], BF16, tag="wq")
    nc.sync.dma_start(wq_sb, wq2)
    # i2/eye gate the first 'pe'-class unit / first transpose; keep them at
    # the head of the in-order sync DMA queue.
    eye = const.tile([128, 128], BF16, tag="eye")
    i2 = const.tile([128, 256], FP8, tag="i2")
    woe_sb = [const.tile([128, E], BF16, tag=f"woe{pc}", name=f"woe{pc}")
              for pc in range(4)]
    bo_sb = const.tile([128, E], F32, tag="bo")
    biasA = const.tile([128, 1], F32, tag="biasA")   # ACT+PE-mask class
    nc.vector.memset(biasA, -30.0)
    biasB = const.tile([128, 1], F32, tag="biasB")   # ACT, post-exp mask
    nc.vector.memset(biasB, 0.0)
    nc.sync.dma_start(i2, i2_d)
    nc.sync.dma_start(eye, eye_d)

    def load_late_consts():
        for pc in range(4):
            nc.sync.dma_start(woe_sb[pc], woe[pc * 128:(pc + 1) * 128, :])
        nc.sync.dma_start(bo_sb, bo_b)

    # resident masks + V (loaded once, interleaved with first projections)
    mbq_res = {c: const.tile([128, QLEN], FP8, tag=f"mq{c}", name=f"mq{c}")
               for c in PE_KCS}
    mbb_res = {c: const.tile([128, QLEN], BF16, tag=f"mb{c}", name=f"mb{c}")
               for c in BF_KCS}
    mba_res = {c: const.tile([128, QLEN], F16, tag=f"ma{c}", name=f"ma{c}")
               for c in DVE_KCS}
    valp_t = [const.tile([128, H * 65], BF16, tag=f"vp{c}", name=f"vp{c}")
              for c in range(KC)]

    def load_kv_masks(lo, hi):
        for c in range(lo, hi):
            nc.sync.dma_start(valp_t[c], valp[c * 128:(c + 1) * 128, :])
            if c in mbq_res:
                nc.sync.dma_start(mbq_res[c], mbq[c * 128:(c + 1) * 128, :])
            if c in mbb_res:
                nc.sync.dma_start(mbb_res[c], mbb[c * 128:(c + 1) * 128, :])
            if c in mba_res:
                nc.sync.dma_start(mba_res[c], mba[c * 128:(c + 1) * 128, :])

    # projections, per pair: kp [128 (h0 rows 0-63, h1 64-127), S] bf16
    kp2 = [kqp.tile([128, S], BF16, tag=f"kp2_{p}", name=f"kp2_{p}")
           for p in range(4)]
    qp2 = [kqp.tile([128, QLEN], BF16, tag=f"qp2_{p}", name=f"qp2_{p}")
           for p in range(4)]

    concatT = [const.tile([128, QLEN], BF16, tag=f"ct{p}", name=f"ct{p}")
               for p in range(4)]

    xs = {}
    copy_idx = [0]

    def proj_load(pair):
        # two half-tiles so early proj chunks start after 0.5MB, not 1MB
        xk0 = xst.tile([128, S // 2], BF16, tag="xka", name=f"xka{pair}")
        nc.gpsimd.dma_start(xk0, xkT[pair * 128:(pair + 1) * 128, 0:S // 2])
        xq = xst.tile([128, QLEN], BF16, tag="xq", name=f"xq{pair}")
        nc.gpsimd.dma_start(xq, xqT[pair * 128:(pair + 1) * 128, :])
        xk1 = xst.tile([128, S // 2], BF16, tag="xkb", name=f"xkb{pair}")
        nc.gpsimd.dma_start(xk1, xkT[pair * 128:(pair + 1) * 128, S // 2:])
        xs[pair] = ((xk0, xk1), xq)

    def _pcopy(dst, src, nm):
        eng = _copy_engine(copy_idx[0])
        copy_idx[0] += 1
        if eng == 'v':
            nc.vector.tensor_copy(dst, src)
        elif eng == 'p':
            nc.gpsimd.tensor_copy(dst, src)
        else:
            nc.scalar.copy(dst, src)

    def proj_chunks(pair):
        """Closures: per 512-col chunk: 2 matmuls (h0/h1) + 1 copy."""
        (xk0, xk1), xq = xs[pair]
        works = []

        def chunk(csrc, coff, w_sb, dst, c, width, nm):
            def run():
                ps = psp.tile([128, 512], F32, tag="ps", name=f"{nm}{c}")
                lo = c * 512 - coff
                nc.tensor.matmul(ps[0:64, 0:width], lhsT=w_sb[0:64, :],
                                 rhs=csrc[0:64, lo:lo + width],
                                 start=True, stop=True)
                nc.tensor.matmul(ps[64:128, 0:width], lhsT=w_sb[64:128, :],
                                 rhs=csrc[64:128, lo:lo + width],
                                 start=True, stop=True)
                _pcopy(dst[:, c * 512:c * 512 + width], ps[:, 0:width], nm)
            return [run]

        for c in range(4):
            works += chunk(xk0, 0, wk_sb, kp2[pair], c, 512, f"k{pair}_")
        for c in range(2):
            works += chunk(xq, 0, wq_sb, qp2[pair], c, 512, f"q{pair}_")
        for c in range(4, 8):
            works += chunk(xk1, S // 2, wk_sb, kp2[pair], c, 512, f"k{pair}_")
        return works

    # ---------------- attention ----------------
    def attn(pair, qw, trickle=(), last=False):
        trickle = list(trickle)
        cx = ctxp.tile([128, 1024], F32, tag="cx", name=f"cx{pair}_{qw}")

        def scores(kc, h2):
            ps = psp.tile([128, 512], F32, tag="ps",
                          name=f"ps{pair}_{qw}_{kc}_{h2}")
            cls = CLS[kc]
            nc.tensor.matmul(ps, lhsT=kp2[pair][h2 * 64:(h2 + 1) * 64,
                                                kc * 128:(kc + 1) * 128],
                             rhs=qp2[pair][h2 * 64:(h2 + 1) * 64,
                                           qw * 512:(qw + 1) * 512],
                             start=True, stop=(cls != 'pe'))
            if cls == 'pe':
                i2v = bass.AP(tensor=i2.tensor, offset=i2.offset,
                              ap=[i2.ap[0], [128, 2], [1, 128]])
                ms = mbq_res[kc][:, qw * 512:(qw + 1) * 512]
                mv = bass.AP(tensor=ms.tensor, offset=ms.offset,
                             ap=[ms.ap[0], [0, 2], [1, 512]])
                nc.tensor.matmul(ps, lhsT=i2v, rhs=mv, start=False, stop=True,
                                 perf_mode=DR)
            return ps

        def expmask(kc, h2, ps):
            cls = CLS[kc]
            if cls == 'dve':
                pti = ptp.tile([128, 512], I16, tag="pt",
                               name=f"pt{pair}_{qw}_{kc}_{h2}")
                ma = mba_res[kc][:, qw * 512:(qw + 1) * 512]
                nc.vector.scalar_tensor_tensor(pti, ps, BBIG, ma,
                                               ALU.add, ALU.mult)
                return pti.bitcast(BF16)
            pt = ptp.tile([128, 512], BF16, tag="pt",
                          name=f"pt{pair}_{qw}_{kc}_{h2}")
            bias = biasA if cls == 'pe' else biasB
            nc.scalar.activation(pt, ps, AF.Exp, bias=bias, scale=0.125)
            if cls in ('pool', 'dvm'):
                ms = mbb_res[kc][:, qw * 512:(qw + 1) * 512]
                if cls == 'pool':
                    nc.gpsimd.tensor_mul(pt, pt, ms)
                else:
                    nc.vector.tensor_mul(pt, pt, ms)
            return pt

        def av(kc, h2, pt):
            if dbg and pair == 0 and qw == 0 and h2 == 0 and kc in (0, 1):
                dt = dbg_p.tile([128, 512], F32, tag=f"dbgpt{kc}")
                nc.vector.tensor_copy(dt, pt)
                nc.sync.dma_start(dbg[f'pt{kc}'], dt)
            h = 2 * pair + h2
            # blocks live bank-aligned at h2*512 + qt*65; a matmul's
            # start=True zeroes the full 2KB psum region, so ONLY the first
            # block of each bank may set it (the pending-zero then covers
            # the other blocks' first accumulation).
            for qt in range(4):
                base = h2 * 512 + qt * 65
                nc.tensor.matmul(
                    cx[:, base:base + 65],
                    lhsT=pt[:, qt * 128:qt * 128 + 128],
                    rhs=valp_t[kc][:, h * 65:(h + 1) * 65],
                    start=(kc == 0 and qt == 0), stop=(kc == KC - 1),
                    skip_group_check=True)

        # software-pipelined half-units (one head each); AV lags by 4 halves
        # so its pt dependency is long satisfied at PE dispatch time.
        from collections import deque
        pend = deque()
        hu = 0
        for kc in range(KC):
            for h2 in range(2):
                ps = scores(kc, h2)
                # pool-masked pt arrives ~2-3us after its exp (Pool queue +
                # 1.1us mult): give those units a deeper AV lag.
                lag = 8 if kc < KC - 5 else 4
                if len(pend) >= lag:
                    av(*pend.popleft())
                    if len(pend) >= lag:
                        av(*pend.popleft())
                pt = expmask(kc, h2, ps)
                pend.append((kc, h2, pt))
                if trickle and hu >= 2 and hu % 2 == 0:
                    trickle.pop(0)()
                hu += 1
        # tail: h0's normalize chain starts while h1's last AVs run, so the
        # cx psum frees as early as possible (it is single-buffered).
        ctn = {}

        def tail_head(h2):
            r = ctn_p.tile([128, 4], F32, tag="rec", name=f"rc{pair}_{qw}_{h2}")
            dn = bass.AP(tensor=cx.tensor, offset=cx.offset + h2 * 512 + 64,
                         ap=[cx.ap[0], [65, 4]])
            with nc.allow_low_precision(reason="softmax denom reciprocal f32"):
                nc.vector.reciprocal(r, dn)
            for qt in range(4):
                t = ctn_p.tile([128, 64], BF16, tag="ctn",
                               name=f"cn{pair}_{qw}_{h2}_{qt}")
                nc.vector.tensor_scalar(
                    t, cx[:, h2 * 512 + qt * 65:h2 * 512 + qt * 65 + 64],
                    r[:, qt:qt + 1], None, ALU.mult)
                ctn[(h2, qt)] = t

        if dbg and pair == 0 and qw == 0:
            dcx = dbg_p.tile([128, 1024], F32, tag="dbgcx")
        while pend:
            kc_, h2_, pt_ = pend.popleft()
            av(kc_, h2_, pt_)
            if kc_ == KC - 1:
                if dbg and pair == 0 and qw == 0 and h2_ == 1:
                    nc.vector.tensor_copy(dcx, cx)
                    nc.sync.dma_start(dbg['cx'], dcx)
                tail_head(h2_)
        for work in trickle:
            work()

        def transp(h2, qt):
            def go():
                tf = psp.tile([128, 512], F32, tag="ps",
                              name=f"tp{pair}_{qw}_{h2}_{qt}")
                tp = tf[0:64, 0:64].bitcast(BF16)
                nc.tensor.transpose(tp, ctn[(h2, qt)], eye)
                dst = concatT[pair][h2 * 64:(h2 + 1) * 64,
                                    qw * 512 + qt * 128:qw * 512 + qt * 128 + 128]
                if last:
                    nc.scalar.copy(dst, tp)   # ACT is idle during the drain
                else:
                    nc.vector.tensor_copy(dst, tp)
            return go

        return [transp(h2, qt) for qt in range(4) for h2 in range(2)]

    def outproj(qts):
        def one(qt):
            def go():
                op = psp.tile([128, 512], F32, tag="ps", name=f"op{qt}")
                for pc in range(4):
                    nc.tensor.matmul(op,
                                     lhsT=concatT[pc][:, qt * 128:(qt + 1) * 128],
                                     rhs=woe_sb[pc],
                                     start=(pc == 0), stop=(pc == 3))
                osb = osb_p.tile([128, E], F32, tag="osb", name=f"osb{qt}")
                nc.vector.scalar_tensor_tensor(osb, op, 1.0, bo_sb,
                                               ALU.mult, ALU.add)
                nc.sync.dma_start(out[qt * 128:(qt + 1) * 128, :], osb)
            return go
        return [one(qt) for qt in qts]

    # ---------------- schedule ----------------
    proj_load(0)
    load_kv_masks(0, 8)
    for work in proj_chunks(0):
        work()
    proj_load(1)
    load_kv_masks(8, KC)
    load_late_consts()
    if dbg:
        nc.sync.dma_start(dbg['kp2'], kp2[0])
        nc.sync.dma_start(dbg['qp2'], qp2[0])
    t00 = attn(0, 0, trickle=proj_chunks(1))
    proj_load(2)
    t01 = attn(0, 1, trickle=t00 + proj_chunks(2))
    t10 = attn(1, 0, trickle=t01)
    proj_load(3)
    t11 = attn(1, 1, trickle=t10 + proj_chunks(3))
    t20 = attn(2, 0, trickle=t11)
    t21 = attn(2, 1, trickle=t20)
    t30 = attn(3, 0, trickle=t21)
    t31 = attn(3, 1, trickle=t30 + outproj(range(4)), last=True)
    ops = outproj(range(4, 8))
    for qt in range(4):
        t31[2 * qt]()      # transp (h2=0, qt)
        t31[2 * qt + 1]()  # transp (h2=1, qt)
        ops[qt]()
    if dbg:
        dct = dbg_p.tile([128, 1024], F32, tag="dbgct")
        nc.vector.tensor_copy(dct, concatT[0])
        nc.sync.dma_start(dbg['ct'], dct)


    ctx.close()


def _prep_inputs(key, query, value, mask, Wq, Wk, Wv, Wo, bo):
    bf16 = ml_dtypes.bfloat16
    fp8 = ml_dtypes.float8_e4m3
    key = np.asarray(key, np.float32)
    query = np.asarray(query, np.float32)
    value = np.asarray(value, np.float32)
    mask = np.asarray(mask)
    Wq = np.asarray(Wq, np.float32)
    Wk = np.asarray(Wk, np.float32)
    Wv = np.asarray(Wv, np.float32)
    Wo = np.asarray(Wo, np.float32)
    bo = np.asarray(bo, np.float32)

    # Wo_eff[e, h*64+u] = sum_d Wo[e, h*64+d] * Wv[d, u]
    wo_eff = np.empty((E, E), np.float32)
    for h in range(H):
        wo_eff[:, h * DH:(h + 1) * DH] = Wo[:, h * DH:(h + 1) * DH] @ Wv
    woe = np.ascontiguousarray(wo_eff.T).astype(bf16)   # [(h,u), e]

    wk2 = np.ascontiguousarray(np.vstack([Wk.T, Wk.T])).astype(bf16)
    wq2 = np.ascontiguousarray(np.vstack([Wq.T, Wq.T])).astype(bf16)

    i2 = np.zeros((128, 256), np.float32)
    i2[:, 0:128] = np.eye(128) * 240.0

    m01 = (mask[0, 0] != 0).astype(np.float32).T  # [k, q] in {0,1}
    common = {
        "wk2": wk2, "wq2": wq2, "woe": woe,
        "bo_b": np.ascontiguousarray(np.broadcast_to(bo, (128, E))).astype(np.float32),
        "eye": np.eye(128, dtype=np.float32).astype(bf16),
        "i2": i2.astype(fp8),
    }
    per_b = {}
    for b in range(B):
        vp = np.ones((S, H, 65), np.float32)
        vp[:, :, :64] = value[b].reshape(S, H, DH)
        per_b[b] = {
            "xkT": np.ascontiguousarray(key[b].T).astype(bf16),
            "valp": np.ascontiguousarray(vp.reshape(S, H * 65)).astype(bf16),
            "qT": query[b].T,
        }
    in_maps = []
    for c in range(N_CORES):
        b, qs = c // 4, (c % 4) * QLEN
        msl = np.ascontiguousarray(m01[:, qs:qs + QLEN])
        in_maps.append({
            "xkT": per_b[b]["xkT"],
            "xqT": np.ascontiguousarray(per_b[b]["qT"][:, qs:qs + QLEN]).astype(bf16),
            "valp": per_b[b]["valp"],
            "mbq": msl.astype(fp8),
            "mbb": msl.astype(bf16),
            "mba": (msl * A16).astype(np.float16),
            **common,
        })
    return in_maps


def get_module():
    if "nc" not in _CACHE:
        _CACHE["nc"] = _build_module()
    return _CACHE["nc"]


def kernel(key, query, value, mask, Wq, Wk, Wv, Wo, bo, **_):
    nc = get_module()
    in_maps = _prep_inputs(key, query, value, mask, Wq, Wk, Wv, Wo, bo)
    res = bass_utils.run_bass_kernel_spmd(
        nc, in_maps, core_ids=list(range(N_CORES)))
    full = np.empty((B, S, E), np.float32)
    for c in range(N_CORES):
        b, qs = c // 4, (c % 4) * QLEN
        full[b, qs:qs + QLEN, :] = res.results[c]["out"]
    return full

